# revision 63
# baseline (speedup 1.0000x reference)
"""Trainium2 Bass kernel for nn_CausalSelfAttention_45200235823551.

Causal self-attention with low-rank key/value encoders:
  D=1024, H=16 heads, HD=64, F=32 freqs, KR=3, VR=192, B=2, S=2048.

Sharding: 8 cores = 2 batches x 4 head-groups. Core i owns batch i//4 and
heads 4*(i%4)..4*(i%4)+3. Each core computes its heads' q/k/v, attention,
and a partial output projection (its heads' rows of Wproj); the host sums
the 4 partials per batch (row-parallel linear unshard).

Per-core layout ("T-major": feature rows on partitions, sequence on free):
  - xT [1024, 2048] bf16, one 512-wide chunk per DMA; chunk 0 streams as
    k-pair pieces on the SWDGE queue (descriptor generation parallel to
    the HWDGE weight loads), consumed kt-major so PE starts ~2.5us earlier
  - wcomb [1024, 768] = [Wq(2x128) | Wk@A key-fold (2x128) | Wv@dec(2x128)]
    BOTH decoders fold into the projection on the host:
    kT = A^T (Wk^T x) = (Wk A)^T x and v_h = x @ (Wv dec_h), so there is
    no separate decode stage and all six weight tiles are full-width.
  - RoPE pair-swap via a 128x128 permutation matmul (PE; chunk 0's rope
    psum borrows the acc ring so chunk 1's projections start underneath),
    then qrot = q*cos + (Pq)*sin on DVE with sign folded into sinT
  - scoresT[sk,sq] = krotT.T @ qrotT per head (K=64)
  - causal N-shrink: diagonal block m only computes columns >= 128m;
    triangular mask is a [128,128] multiply on the first 128 columns
  - v path: vT [hd, seq] from the projection is PE-transposed per
    128-seq block into v_sb [seq, v_h(64)|1] x 4 heads = 260 cols, with
    one grouped-AP DVE copy per block; the denominator ones-columns are
    written once by a strided memset.
  - attn@v accumulates [sq-subtile(128), 65] per (head, subtile): each pass
    is lhsT=et[:, subtile], rhs=[v|1] with N=65 (vs 512 for the v-stationary
    form) -- the PE cost model charges N rows per pass, so this nearly
    halves attention-value cost. Column 64 accumulates the softmax
    denominator for free.
  - normalize: per-partition reciprocal of the den column + tensor_scalar
    multiply into y2 [sq,128] (two heads side by side); y2 is transposed
    back to [hd, sq] via DMA-transpose (chunks 0-2, free) or PE transpose
    (last chunk, low latency) for the output projection.
  - partial projection: yout[sq,:] = yn.T @ Wproj_rows, bf16 partials
  - emission is software-pipelined: chunk c's attention interleaves with
    chunk c+1's projections; output projections are deferred into the
    final chunk's window, which is Activation(exp)-bound.
"""

import sys

import numpy as np

sys.path.insert(0, "/opt/trn_rl_repo")

import ml_dtypes

D, H, HD = 1024, 16, 64
F, KR, VR = 32, 3, 192
B, S = 2, 2048
NCORE = 8
CH = 512          # sq chunk width
NCH = S // CH     # 4
BLK = 128         # sk block
VW = 260          # v_sb per-block: 4 heads x [v(64)|ones(1)]
ROPE_BASE = 10000.0

_COMPILED = {}


def _build_bass():
    import concourse.bass as bass
    import concourse.tile as tile
    from concourse import mybir
    from contextlib import ExitStack

    BF = mybir.dt.bfloat16
    F32 = mybir.dt.float32
    AF = mybir.ActivationFunctionType

    nc = bass.Bass()
    xt = nc.dram_tensor("xt", [D, S], BF, kind="ExternalInput")
    wcomb = nc.dram_tensor("wcomb", [D, 768], BF, kind="ExternalInput")
    permw = nc.dram_tensor("permw", [128, 128], BF, kind="ExternalInput")
    wproj = nc.dram_tensor("wproj", [256, D], BF, kind="ExternalInput")
    cosT = nc.dram_tensor("cosT", [128, S], BF, kind="ExternalInput")
    sinT = nc.dram_tensor("sinT", [128, S], BF, kind="ExternalInput")
    trim = nc.dram_tensor("trim", [128, 128], BF, kind="ExternalInput")
    ident = nc.dram_tensor("ident", [128, 128], BF, kind="ExternalInput")
    yout = nc.dram_tensor("yout", [S, D], BF, kind="ExternalOutput")

    with ExitStack() as ctx:
        tc = ctx.enter_context(tile.TileContext(nc))
        consts = ctx.enter_context(tc.tile_pool(name="consts", bufs=1))
        bigs = ctx.enter_context(tc.tile_pool(name="bigs", bufs=1))
        xpool = ctx.enter_context(tc.tile_pool(name="xpool", bufs=2))
        tmps = ctx.enter_context(tc.tile_pool(name="tmps", bufs=4))
        ynpool = ctx.enter_context(tc.tile_pool(name="ynpool", bufs=1))
        epool = ctx.enter_context(tc.tile_pool(name="epool", bufs=26))
        otpool = ctx.enter_context(tc.tile_pool(name="otpool", bufs=7))
        y2pool = ctx.enter_context(tc.tile_pool(name="y2pool", bufs=10))
        rcpool = ctx.enter_context(tc.tile_pool(name="rcpool", bufs=8))
        mmpool = ctx.enter_context(tc.tile_pool(name="mmpool", bufs=2, space="PSUM"))
        sppool = ctx.enter_context(tc.tile_pool(name="sppool", bufs=2, space="PSUM"))
        accpool = ctx.enter_context(tc.tile_pool(name="accpool", bufs=2, space="PSUM"))

        # ---- input prefetch + constants, ordered for fastest PE start ----
        def load_x(c, eng=None):
            cs = slice(c * CH, (c + 1) * CH)
            t = xpool.tile([128, 8 * CH], BF, tag="xts", name="xts")
            (eng or nc.sync).dma_start(
                out=t[:, :].rearrange("p (k j) -> p k j", k=8, j=CH),
                in_=xt[:, cs].rearrange("(k p) j -> p k j", k=8, p=128))
            return t

        # chunk 0 loads in two 256-col halves (half-major tile layout) with
        # weight tiles interleaved: the kt-major first-half matmuls below
        # start after just w0 + the first half, ~3us before the full chunk
        # could land on the serialized DMA engines
        xts0 = xpool.tile([128, 8 * CH], BF, tag="xts", name="xts0")
        wcomb_sb = []

        def load_w(kt, split=False):
            t = consts.tile([128, 768], BF, tag=f"wcomb{kt}")
            if split:
                nc.sync.dma_start(out=t[:, 0:128],
                                  in_=wcomb[kt * 128:(kt + 1) * 128, 0:128])
                nc.sync.dma_start(out=t[:, 128:768],
                                  in_=wcomb[kt * 128:(kt + 1) * 128, 128:768])
            else:
                nc.sync.dma_start(out=t,
                                  in_=wcomb[kt * 128:(kt + 1) * 128, :])
            wcomb_sb.append(t)

        def load_x0_piece(k0, k1, h, eng=None):
            # k-pair piece of chunk-0 half h: feeds the kt-major c0 loop as
            # soon as each small transfer lands. Issued on the gpsimd SWDGE
            # queue so descriptor generation runs parallel to the HWDGE
            # weight-tile generation (the DMA device serializes transfers,
            # small early pieces start PE sooner).
            (eng or nc.gpsimd).dma_start(
                out=xts0[:, h * 2048:(h + 1) * 2048].rearrange(
                    "p (k j) -> p k j", k=8, j=256)[:, k0:k1, :],
                in_=xt[:, h * 256:(h + 1) * 256].rearrange(
                    "(k p) j -> p k j", k=8, p=128)[:, k0:k1, :])

        load_w(0)
        load_x0_piece(0, 2, 0)
        load_w(1)
        load_x0_piece(2, 4, 0)
        load_w(2)
        load_x0_piece(4, 6, 0)
        load_w(3)
        load_x0_piece(6, 8, 0)
        for kt in range(4, 8):
            load_w(kt)
        load_x0_piece(0, 4, 1, eng=nc.sync)
        load_x0_piece(4, 8, 1, eng=nc.sync)
        xts_pending = [xts0]
        perm_sb = consts.tile([128, 128], BF, tag="perm")
        nc.sync.dma_start(out=perm_sb, in_=permw[:, :])
        cos_sb = consts.tile([128, S], BF, tag="cos")
        nc.sync.dma_start(out=cos_sb, in_=cosT[:, :])
        sin_sb = consts.tile([128, S], BF, tag="sin")
        nc.sync.dma_start(out=sin_sb, in_=sinT[:, :])
        tri_sb = consts.tile([128, 128], BF, tag="tri")
        nc.sync.dma_start(out=tri_sb, in_=trim[:, :])
        id_sb = consts.tile([128, 128], BF, tag="ident")
        nc.sync.dma_start(out=id_sb, in_=ident[:, :])
        wp_sb = []
        for i in range(2):
            t = consts.tile([128, D], BF, tag=f"wp{i}")
            nc.sync.dma_start(out=t, in_=wproj[i * 128:(i + 1) * 128, :])
            wp_sb.append(t)

        # persistent per-core tensors
        qrot = [bigs.tile([128, S], BF, tag=f"qrot{i}", name=f"qrot{i}")
                for i in range(2)]
        krot = [bigs.tile([128, S], BF, tag=f"krot{i}", name=f"krot{i}")
                for i in range(2)]
        v_sb = bigs.tile([128, 16 * VW], BF, tag="v")
        vT = [bigs.tile([128, S], BF, tag=f"vT{i}", name=f"vT{i}")
              for i in range(2)]
        # denominator ones-columns (col 64 of each head's 65-wide slot),
        # written once; the per-block v copies never touch them
        nc.gpsimd.memset(v_sb[:, 64:16 * VW:65], 1.0)

        def proj_copy(ct, ps, cs, xvab, xvb2, c):
            # q/k drains stay on DVE (latency-critical for rope); the xv
            # drains go to Act, which has slack until the late windows
            if ct == 0:
                nc.vector.tensor_copy(qrot[0][:, cs], ps)
            elif ct == 1:
                nc.vector.tensor_copy(qrot[1][:, cs], ps)
            elif ct == 2:
                nc.vector.tensor_copy(krot[0][:, cs], ps)
            elif ct == 3:
                nc.vector.tensor_copy(krot[1][:, cs], ps)
            elif ct == 4:
                (nc.scalar.copy if c < 3 else
                 nc.vector.tensor_copy)(vT[0][:, cs], ps)
            else:
                (nc.scalar.copy if c < 3 else
                 nc.vector.tensor_copy)(vT[1][:, cs], ps)

        def phase_a_qk(c, xv):
            """Generator: q/k projection + rope for chunk c. Bursting this
            at the start of the previous window starts the next chunk's
            scores (and so the Act exp stream) several us earlier."""
            cs = slice(c * CH, (c + 1) * CH)
            xts = xts_pending.pop(0)
            if c + 1 < NCH:
                xts_pending.append(load_x(c + 1))
            for ct in range(4):
                ps = mmpool.tile([128, CH], F32, tag="mm", name="mm")
                for kt in range(8):
                    nc.tensor.matmul(
                        ps,
                        lhsT=wcomb_sb[kt][:, ct * 128:(ct + 1) * 128],
                        rhs=xts[:, kt * CH:(kt + 1) * CH],
                        start=(kt == 0), stop=(kt == 7))
                proj_copy(ct, ps, cs, None, None, c)
                yield
            for tt in (qrot[0], qrot[1], krot[0], krot[1]):
                pq = mmpool.tile([128, CH], F32, tag="mm", name="mm")
                nc.tensor.matmul(pq, lhsT=perm_sb, rhs=tt[:, cs],
                                 start=True, stop=True)
                t1 = tmps.tile([128, CH], BF, tag="t1", name="t1")
                nc.vector.tensor_mul(t1, pq, sin_sb[:, cs])
                t2 = tmps.tile([128, CH], BF, tag="t2", name="t2")
                nc.vector.tensor_mul(t2, tt[:, cs], cos_sb[:, cs])
                nc.vector.tensor_add(tt[:, cs], t1, t2)
                yield
            xv.append(xts)

        def phase_a_v(c, xv):
            """Generator: folded-v projection (whv = Wv @ value_decoder is
            folded into wcomb on the host) + per-block PE transposes from
            [hd, seq] into v_sb's [seq, v|1] layout."""
            xts = xv.pop(0)
            cs = slice(c * CH, (c + 1) * CH)
            for ct in range(4, 6):
                ps = mmpool.tile([128, CH], F32, tag="mm", name="mm")
                for kt in range(8):
                    nc.tensor.matmul(
                        ps,
                        lhsT=wcomb_sb[kt][:, ct * 128:(ct + 1) * 128],
                        rhs=xts[:, kt * CH:(kt + 1) * CH],
                        start=(kt == 0), stop=(kt == 7))
                proj_copy(ct, ps, cs, None, None, c)
                yield
            for j in range(4):
                sb = 4 * c + j
                js = slice(sb * BLK, (sb + 1) * BLK)
                vtp = mmpool.tile([128, 256], BF, tag="mm", name="vtp")
                nc.tensor.transpose(vtp[:, 0:128], vT[0][:, js], id_sb)
                nc.tensor.transpose(vtp[:, 128:256], vT[1][:, js], id_sb)
                # all four heads' 64-col pieces land at their 65-wide slots
                out = v_sb[:, sb * VW:(sb + 1) * VW].rearrange(
                    "p (g q) -> p g q", g=4, q=65)[:, :, 0:64]
                nc.vector.tensor_copy(
                    out, vtp.rearrange("p (g q) -> p g q", g=4, q=64))
                yield

        def phase_a_units(c):
            """Chunk-0 phase A (runs alone): kt-major halves so matmuls
            chase the streamed x/w DMA pieces."""
            cs = slice(c * CH, (c + 1) * CH)
            xts = xts_pending.pop(0)
            if c + 1 < NCH:
                xts_pending.append(load_x(c + 1))
            if c == 0:
                # 6 concurrent psum chains (mm ring + borrowed score-tile
                # halves, idle this early); kt-major over the first x half
                # so each weight tile is consumed as its DMA lands
                pss = [mmpool.tile([128, CH], F32, tag="mm", name="mm")
                       for _ in range(2)]
                spa = sppool.tile([128, 2 * CH], F32, tag="sp", name="sp")
                spb = sppool.tile([128, 2 * CH], F32, tag="sp", name="sp")
                pss += [spa[:, 0:CH], spa[:, CH:2 * CH],
                        spb[:, 0:CH], spb[:, CH:2 * CH]]
                for kt in range(8):
                    for ct in range(6):
                        nc.tensor.matmul(
                            pss[ct][:, 0:256],
                            lhsT=wcomb_sb[kt][:, ct * 128:(ct + 1) * 128],
                            rhs=xts0[:, kt * 256:(kt + 1) * 256],
                            start=(kt == 0), stop=(kt == 7))
                    yield
                for kt in range(8):
                    for ct in range(6):
                        nc.tensor.matmul(
                            pss[ct][:, 256:512],
                            lhsT=wcomb_sb[kt][:, ct * 128:(ct + 1) * 128],
                            rhs=xts0[:, 2048 + kt * 256:2048 + (kt + 1) * 256],
                            start=(kt == 0), stop=(kt == 7))
                    yield
                for ct in (0, 2, 1, 3):
                    tt = (qrot[0], qrot[1], krot[0], krot[1])[ct]
                    nc.vector.tensor_copy(tt[:, c * CH:c * CH + 256],
                                          pss[ct][:, 0:256])
                    nc.scalar.copy(tt[:, c * CH + 256:(c + 1) * CH],
                                   pss[ct][:, 256:512])
                    # rope psum borrows the acc ring (idle until the first
                    # attn@v): keeps the mm ring free so chunk 1's projection
                    # chains start underneath chunk 0's rope drain
                    pq = accpool.tile([128, CH], F32, tag="acc", name="pq")
                    nc.tensor.matmul(pq, lhsT=perm_sb, rhs=tt[:, cs],
                                     start=True, stop=True)
                    t1 = tmps.tile([128, CH], BF, tag="t1", name="t1")
                    nc.vector.tensor_mul(t1, pq, sin_sb[:, cs])
                    t2 = tmps.tile([128, CH], BF, tag="t2", name="t2")
                    nc.vector.tensor_mul(t2, tt[:, cs], cos_sb[:, cs])
                    nc.vector.tensor_add(tt[:, cs], t1, t2)
                    yield
                for ct in range(4, 6):
                    proj_copy(ct, pss[ct], cs, None, None, c)
                yield
            # v transposes: per 128-seq block, [hd, seq] -> [seq, v|1]
            for j in range(4):
                sb = 4 * c + j
                js = slice(sb * BLK, (sb + 1) * BLK)
                vtp = mmpool.tile([128, 256], BF, tag="mm", name="vtp")
                nc.tensor.transpose(vtp[:, 0:128], vT[0][:, js], id_sb)
                nc.tensor.transpose(vtp[:, 128:256], vT[1][:, js], id_sb)
                out = v_sb[:, sb * VW:(sb + 1) * VW].rearrange(
                    "p (g q) -> p g q", g=4, q=65)[:, :, 0:64]
                nc.vector.tensor_copy(
                    out, vtp.rearrange("p (g q) -> p g q", g=4, q=64))
                yield

        def scores_units(c, h, ets):
            """Generator: scores + exp (+ causal mask) for head h, chunk c.
            Appends et tiles to ets; one psum tile per yield."""
            hp, hl = h // 2, h % 2
            rows = slice(hl * 64, (hl + 1) * 64)
            kk, qq = krot[hp], qrot[hp]
            cs = slice(c * CH, (c + 1) * CH)
            for g in range(2 * c):   # full block pairs
                sp = sppool.tile([128, 2 * CH], F32, tag="sp", name="sp")
                for i in range(2):
                    blk = 2 * g + i
                    nc.tensor.matmul(
                        sp[:, i * CH:(i + 1) * CH],
                        lhsT=kk[rows, blk * BLK:(blk + 1) * BLK],
                        rhs=qq[rows, cs], start=True, stop=True)
                et = epool.tile([128, 2 * CH], BF, tag="et", name="et")
                nc.scalar.activation(et, sp, AF.Exp, scale=0.125)
                ets.append(et)
                yield
            # diagonal blocks, N-shrunk: m covers cols [128m:512] of the chunk
            spd0 = sppool.tile([128, 2 * CH], F32, tag="sp", name="sp")
            nc.tensor.matmul(
                spd0[:, 0:512],
                lhsT=kk[rows, (4 * c) * BLK:(4 * c + 1) * BLK],
                rhs=qq[rows, cs], start=True, stop=True)
            nc.tensor.matmul(
                spd0[:, 512:896],
                lhsT=kk[rows, (4 * c + 1) * BLK:(4 * c + 2) * BLK],
                rhs=qq[rows, c * CH + 128:(c + 1) * CH], start=True, stop=True)
            etd0 = epool.tile([128, 2 * CH], BF, tag="et", name="et")
            nc.scalar.activation(etd0[:, 0:896], spd0[:, 0:896],
                                 AF.Exp, scale=0.125)
            trim_eng = nc.vector
            trim_eng.tensor_mul(etd0[:, 0:128], etd0[:, 0:128], tri_sb)
            trim_eng.tensor_mul(etd0[:, 512:640], etd0[:, 512:640], tri_sb)
            ets.append(etd0)
            yield
            spd1 = sppool.tile([128, 2 * CH], F32, tag="sp", name="sp")
            nc.tensor.matmul(
                spd1[:, 0:256],
                lhsT=kk[rows, (4 * c + 2) * BLK:(4 * c + 3) * BLK],
                rhs=qq[rows, c * CH + 256:(c + 1) * CH], start=True, stop=True)
            nc.tensor.matmul(
                spd1[:, 256:384],
                lhsT=kk[rows, (4 * c + 3) * BLK:(4 * c + 4) * BLK],
                rhs=qq[rows, c * CH + 384:(c + 1) * CH], start=True, stop=True)
            etd1 = epool.tile([128, 2 * CH], BF, tag="et", name="et")
            nc.scalar.activation(etd1[:, 0:384], spd1[:, 0:384],
                                 AF.Exp, scale=0.125)
            trim_eng.tensor_mul(etd1[:, 0:128], etd1[:, 0:128], tri_sb)
            trim_eng.tensor_mul(etd1[:, 256:384], etd1[:, 256:384], tri_sb)
            ets.append(etd1)
            yield

        def et_col(c, ets, b, t):
            """(tile, col0) for block b's et columns of sq-subtile t."""
            if b < 4 * c:
                return ets[b // 2], (b % 2) * CH + t * BLK
            m = b - 4 * c
            if m == 0:
                return ets[-2], t * BLK
            if m == 1:
                return ets[-2], 512 + (t - 1) * BLK
            if m == 2:
                return ets[-1], (t - 2) * BLK
            return ets[-1], 256

        def attnv_units(c, h, ets, y2s, yn):
            """attn@v for head h, chunk c: per sq-subtile t accumulate
            [sq(128), 65] over sk blocks (rhs = [v|ones], N=65); then
            normalize into y2s[t] (cols hl*64:..). Groups ~8 matmuls/yield."""
            hp, hl = h // 2, h % 2
            acc = accpool.tile([128, 4 * 65], F32, tag="acc", name="acc")
            nmm = 0
            for t in range(4):
                T = 4 * c + t
                a = acc[:, t * 65:(t + 1) * 65]
                for b in range(T + 1):
                    et, col0 = et_col(c, ets, b, t)
                    nc.tensor.matmul(
                        a, lhsT=et[:, col0:col0 + BLK],
                        rhs=v_sb[:, b * VW + h * 65:b * VW + (h + 1) * 65],
                        start=(b == 0), stop=(b == T))
                    nmm += 1
                    if nmm % 8 == 0:
                        yield
            # normalize: y2[t][:, hl*64:...] = acc_y * 1/acc_den
            last = (c == NCH - 1 and h == 3)
            for t in range(4):
                a = acc[:, t * 65:(t + 1) * 65]
                rc = rcpool.tile([128, 1], F32, tag="rc", name="rc")
                nc.vector.reciprocal(rc, a[:, 64:65])
                if hl == 0:
                    y2 = y2pool.tile([128, 128], BF, tag="y2", name="y2")
                    y2s.append(y2)
                nc.vector.tensor_scalar_mul(
                    y2s[t][:, hl * 64:(hl + 1) * 64], a[:, 0:64],
                    rc[:, 0:1])
                if hl == 1:
                    # both heads of the pair are in: transpose back to
                    # [hd, sq] for the output projection
                    if c < NCH - 1:
                        nc.sync.dma_start_transpose(
                            out=yn[hp][:, t * BLK:(t + 1) * BLK], in_=y2s[t])
                    else:
                        ytp = mmpool.tile([128, 128], BF, tag="mm",
                                          name="ytp")
                        nc.tensor.transpose(ytp, y2s[t], id_sb)
                        if t % 2 == 0:
                            nc.scalar.copy(
                                yn[hp][:, t * BLK:(t + 1) * BLK], ytp)
                        else:
                            nc.vector.tensor_copy(
                                yn[hp][:, t * BLK:(t + 1) * BLK], ytp)
                    if t == 3:
                        y2s.clear()
                if last:
                    yield ("sub", t)
            if not last:
                yield

        def b_units(c, yn):
            """Attention for chunk c: pair head h's scores with head h-1's
            attn@v so PE alternates between them while exps drain. Yields
            the head index after that head's normalize, else None."""
            prev = None
            y2s = []
            if c == 0:
                # cold start: lag attn@v two heads behind scores -- paired
                # one-behind it would head-of-line-block the PE queue while
                # the Act pipe is still filling
                etss = []
                for h in range(4):
                    ets = []
                    sg = scores_units(c, h, ets)
                    ag = (attnv_units(c, h - 2, etss[h - 2], y2s, yn)
                          if h >= 2 else None)
                    for _ in sg:
                        yield None
                        if ag is not None and next(ag, "done") != "done":
                            yield None
                    if ag is not None:
                        for u in ag:
                            yield u
                        yield h - 2
                    etss.append(ets)
                for h in (2, 3):
                    for u in attnv_units(c, h, etss[h], y2s, yn):
                        yield u
                    yield h
                return
            for h in range(4):
                ets = []
                sg = scores_units(c, h, ets)
                ag = attnv_units(c, prev[0], prev[1], y2s, yn) if prev \
                    else None
                for _ in sg:
                    yield None
                    if ag is not None and next(ag, "done") != "done":
                        yield None
                if ag is not None:
                    for _ in ag:
                        yield None
                    yield prev[0]
                prev = (h, ets)
            for u in attnv_units(c, prev[0], prev[1], y2s, yn):
                yield u
            yield prev[0]

        def out_proj_units(c, yn, split_dma=False):
            for j in range(4):
                sb = 4 * c + j
                js = slice(j * BLK, (j + 1) * BLK)
                ot = otpool.tile([128, D], BF, tag="ot", name="ot")
                for n in range(2):
                    if split_dma:
                        # scores are done: borrow the freed score-psum ring
                        # (4 banks) so the last projections never wait on a
                        # copy to drain a 2-deep mm slot
                        pp = sppool.tile([128, CH], F32, tag="sp", name="pp")
                    else:
                        pp = mmpool.tile([128, CH], F32, tag="mm", name="mm")
                    nc.tensor.matmul(pp, lhsT=yn[0][:, js],
                                     rhs=wp_sb[0][:, n * CH:(n + 1) * CH],
                                     start=True, stop=False)
                    nc.tensor.matmul(pp, lhsT=yn[1][:, js],
                                     rhs=wp_sb[1][:, n * CH:(n + 1) * CH],
                                     start=False, stop=True)
                    ns = slice(n * CH, (n + 1) * CH)
                    if n == 1 and split_dma:
                        nc.scalar.copy(ot[:, ns], pp)
                    else:
                        nc.vector.tensor_copy(ot[:, ns], pp)
                    if split_dma:
                        # spread the final stores across both DGE paths so
                        # the single HWDGE device isn't the tail's drain
                        eng = nc.gpsimd if j < 2 else nc.sync
                        eng.dma_start(
                            out=yout[sb * BLK:(sb + 1) * BLK, ns],
                            in_=ot[:, ns])
                    yield
                if not split_dma:
                    nc.sync.dma_start(
                        out=yout[sb * BLK:(sb + 1) * BLK, :], in_=ot)

        # chunk 0 phase A runs alone (nothing to overlap with yet).
        # Output projections for chunks 0..2 are all deferred into the last
        # chunk's attention window: that window is Activation-bound (no next
        # phase A left to interleave), so it needs the PE filler the most.
        yns = [[ynpool.tile([128, CH], BF, tag=f"yn{c}_{i}", name="yn")
                for i in range(2)] for c in range(NCH)]
        gbs = [b_units(c, yns[c]) for c in range(NCH)]
        # Drive chunk-0 phase A, pulling the first scores ahead of the
        # xv/v-decode units: yields 17/18 are the q0/k0 rope units, after
        # which head 0's (and h1's first) score tiles can be emitted so the
        # Act exp stream starts ~2.5us earlier. No more than 3 pulls: the
        # 4th would emit attn@v, which reads v_sb written by the v-decode
        # still behind it in the in-order PE queue.
        for _ in phase_a_units(0):
            pass
        xvq = []

        def phase_a_chain(c):
            yield from phase_a_qk(c, xvq)
            yield from phase_a_v(c, xvq)

        gops = [out_proj_units(cc, yns[cc]) for cc in range(NCH - 1)]
        nxt = {}
        for c in range(NCH - 1):
            ga = nxt.pop(c + 1, None) or phase_a_chain(c + 1)
            a_done = False
            acc = 0.0
            acc2 = 0.0
            ga2 = None
            nb = 0
            ratio = 14.0 / (16 * c + 20)
            if c == 0:
                # cold start: chunk 0's rope (DVE) gates the first scores;
                # give PE two ready projection chains to chew first
                next(ga, "done")
                next(ga, "done")
            for marker in gbs[c]:
                acc += ratio
                nb += 1
                while acc >= 1.0 and not a_done:
                    if next(ga, "done") == "done":
                        a_done = True
                    else:
                        acc -= 1.0
                # once the next phase A is fully in, pull the next chunk's
                # attention forward: its exp stream lags the PE otherwise.
                # Also start chunk c+2's projection chain (depth-2 pipeline)
                # so later chunks' scores -- and their exps -- shift earlier,
                # filling Act's early idle instead of its late backlog.
                if a_done:
                    if c + 2 < NCH and acc2 > -1.0:
                        if ga2 is None:
                            ga2 = phase_a_chain(c + 2)
                            nxt[c + 2] = ga2
                        acc2 += 0.3
                        while acc2 >= 1.0:
                            acc2 -= 1.0
                            if next(ga2, "done") == "done":
                                acc2 = -2.0
                    next(gbs[c + 1], "done")
                    if c == NCH - 2:
                        next(gbs[c + 1], "done")
            while not a_done:
                if next(ga, "done") == "done":
                    a_done = True
        gfin = out_proj_units(NCH - 1, yns[-1], split_dma=True)
        acc = 0.0
        pace = True
        for marker in gbs[NCH - 1]:
            if isinstance(marker, tuple):
                # last head's subtile t normalized: immediately emit the
                # final projection for that subtile (2 units)
                next(gfin, None)
                next(gfin, None)
                continue
            if marker == 2:
                # keep the DVE queue clear for the last head's normalize:
                # its divide gates the final output projection
                pace = False
            acc += 0.5 if pace else 0.0
            while acc >= 1.0 and gops:
                if next(gops[0], "done") == "done":
                    gops.pop(0)
                else:
                    acc -= 1.0
        for ga in gops:
            for _ in ga:
                pass
        for _ in gfin:
            pass

    _split_dma_waits(nc, mybir)
    return nc


def _split_dma_waits(nc, mybir):
    """This container's walrus rejects instructions whose 64B encoding lacks
    room for their sem waits ("Too many sync wait commands"): DMAs and NoOps
    hold 1 wait, matmuls 2. Hoist excess waits onto a chain of single-wait
    NoOps in the same engine stream directly before the instruction — the
    sequencer blocks on each, which is semantically identical."""
    cap = {}
    f = nc.m.functions[0]
    blocks = f.body if hasattr(f, "body") else f.blocks
    n = 0
    for blk in blocks:
        insts = list(blk.instructions)
        out = []
        changed = False
        for inst in insts:
            si = inst.sync_info
            tn = type(inst).__name__
            limit = cap.get(tn, 1)
            if si is not None and si.on_wait and len(si.on_wait) > limit:
                waits = list(si.on_wait)
                keep = waits[-limit:]
                for w in waits[:-limit]:
                    nop = mybir.InstNoOp(name=f"I-dmaw-{n}")
                    n += 1
                    nop.engine = inst.engine
                    nop.sync_info = mybir.SyncInfo(on_wait=[w], on_update=[])
                    nc.register_instruction(nop)
                    out.append(nop)
                inst.sync_info = mybir.SyncInfo(
                    on_wait=keep, on_update=list(si.on_update or []))
                changed = True
            out.append(inst)
        if changed:
            if hasattr(blk, "set_instructions"):
                blk.set_instructions(out)
            else:
                try:
                    blk.instructions = out
                except Exception:
                    blk.instructions[:] = out
    return nc


def _host_inputs(x, Wq, Wk, Wv, key_decoder, value_decoder, Wproj):
    bf16 = ml_dtypes.bfloat16
    x = np.asarray(x, np.float32)
    Wq = np.asarray(Wq, np.float32)
    Wk = np.asarray(Wk, np.float32)
    Wv = np.asarray(Wv, np.float32)
    key_decoder = np.asarray(key_decoder, np.float32)
    value_decoder = np.asarray(value_decoder, np.float32)
    Wproj = np.asarray(Wproj, np.float32)

    xts = [np.ascontiguousarray(x[b].T).astype(bf16) for b in range(B)]

    half = HD // 2
    freq = 1.0 / (ROPE_BASE ** (np.arange(half, dtype=np.float32) / half))
    th = np.outer(np.arange(S, dtype=np.float32), freq)  # [S, 32]
    cos, sin = np.cos(th), np.sin(th)
    rows = np.arange(128)
    fidx = (rows % 64) // 2
    cosT = cos[:, fidx].T.astype(bf16)                       # [128, S]
    sgn = np.where(rows % 2 == 0, -1.0, 1.0)[:, None]
    sinT = (sin[:, fidx].T * sgn).astype(bf16)

    p = np.arange(128)
    permw = np.zeros((128, 128), np.float32)
    permw[p, p ^ 1] = 1.0       # pair swap
    permw = permw.astype(bf16)

    trim = (p[:, None] <= p[None, :]).astype(np.float32).astype(bf16)
    ident = np.eye(128, dtype=np.float32).astype(bf16)

    Wq4 = Wq.reshape(D, H, HD)
    br, bi = key_decoder[..., 0], key_decoder[..., 1]  # [F, H, KR]

    in_maps = []
    for core in range(NCORE):
        b, hg = core // 4, core % 4
        hh = [4 * hg + i for i in range(4)]

        wq0 = Wq4[:, [hh[0], hh[1]], :].reshape(D, 128)
        wq1 = Wq4[:, [hh[2], hh[3]], :].reshape(D, 128)
        # key decoder folded into Wk: kT = A^T Wk^T x = (Wk A)^T x
        kf = []
        for hp in range(2):
            A = np.zeros((192, 128), np.float32)
            for hl, h in enumerate((hh[2 * hp], hh[2 * hp + 1])):
                for f in range(F):
                    for r in range(KR):
                        A[f * 6 + r * 2 + 0, hl * 64 + 2 * f] = br[f, h, r]
                        A[f * 6 + r * 2 + 1, hl * 64 + 2 * f] = -bi[f, h, r]
                        A[f * 6 + r * 2 + 0, hl * 64 + 2 * f + 1] = bi[f, h, r]
                        A[f * 6 + r * 2 + 1, hl * 64 + 2 * f + 1] = br[f, h, r]
            kf.append(Wk @ A)    # [D, 128]
        # value decoder folded into the projection: v_h = x @ (Wv @ dec_h)
        whv = np.concatenate(
            [Wv @ value_decoder[h] for h in hh], axis=1)        # [D, 256]
        wcomb = np.concatenate(
            [wq0, wq1, kf[0], kf[1], whv], axis=1).astype(bf16)  # [D, 768]

        wproj_my = np.concatenate(
            [Wproj[h * 64:(h + 1) * 64, :] for h in hh], axis=0).astype(bf16)

        in_maps.append({
            "xt": xts[b], "wcomb": wcomb, "permw": permw,
            "wproj": wproj_my, "cosT": cosT, "sinT": sinT, "trim": trim,
            "ident": ident,
        })
    return in_maps


def kernel(x, Wq, Wk, Wv, key_decoder, value_decoder, Wproj):
    from concourse.bass_utils import run_bass_kernel_spmd

    if "nc" not in _COMPILED:
        _COMPILED["nc"] = _build_bass()
    nc = _COMPILED["nc"]

    in_maps = _host_inputs(x, Wq, Wk, Wv, key_decoder, value_decoder, Wproj)
    import time as _time
    t0 = _time.time()
    res = run_bass_kernel_spmd(nc, in_maps, list(range(NCORE)))
    _COMPILED["exec_wall_ns"] = (_time.time() - t0) * 1e9
    _COMPILED["last_result"] = res
    out = np.zeros((B, S, D), np.float32)
    for core in range(NCORE):
        out[core // 4] += res.results[core]["yout"].astype(np.float32)
    return out
